# revision 16
# baseline (speedup 1.0000x reference)
# DCN CrossLayer kernel for Trainium2 (8 NeuronCores, data-parallel over batch).
#
# Reference computation (per example row x of length D, L=3 layers):
#   cross = x
#   for i in range(L):
#       s_i   = <cross, W_i>                  (scalar per example)
#       cross = x * s_i + bias_i + cross
#
# Algebraic collapse: cross_i = a_i * x + B_i with per-example scalar a_i and
# batch-independent vector B_i = sum_{j<i} bias_j.  Then
#   s_i     = a_i * t_i + c_i,   t_i = <x, W_i>,  c_i = <B_i, W_i>
#   a_{i+1} = a_i * (1 + t_i) + c_i
#   out     = a_L * x + B_L
# so the device kernel only needs the three dot products t_i = <x, W_i>,
# a tiny per-row recurrence, and one per-row scale of x.  c_i and B_L are
# computed on the host (they do not depend on the batch).
#
# Measured HW facts this version is tuned around (perfetto):
#   - 16 DMA queues x ~25 GB/s each => ~400 GB/s/core aggregate; descriptor
#     cost scales with bytes, so fp16 I/O (4+4 MiB/core) floors DMA at ~21us
#   - PE matmul cost is ~flat (~215ns + 152ns ldweights) for N <= 512, so
#     dot-groups are 512 rows
#   - gpsimd partition_broadcast has multi-us ucode dispatch latency, so a3
#     is broadcast by a PE ones-matmul into PSUM instead, placed mid-way
#     through the NEXT group's dot matmuls (its operand is ready by then,
#     so the PE stream never stalls and ys can read pb straight from PSUM)
#   - the +1s ride the PSUM accumulation (ones-column matmul); recurrence
#     is one ACT pull of U0 plus two DVE muls reading U1/U2 straight from
#     PSUM at partitions 32/64 (quadrant rule; one PSUM operand per op)
#   - ys/store are emitted per 256-row half (last group: 128-row quarters
#     to shorten the end-of-kernel drip) with a one-group skew, so output
#     DMAs interleave with remaining input DMAs on the shared queues
import os
from contextlib import ExitStack

import numpy as np

import concourse.bacc as bacc
import concourse.bass as bass
import concourse.tile as tile
from concourse import mybir
from concourse.bass_utils import run_bass_kernel_spmd

B, D, L = 16384, 1024, 3
N_CORES = 8
ROWS = B // N_CORES  # rows per core
P = 128
KCH = D // P  # 8 d-chunks of 128
GROUPS = 4
G = ROWS // GROUPS  # 512 rows per dot-group
LPAD = 65  # zero-padded stationary width; layer l at column 32*l

F32 = mybir.dt.float32
F16 = mybir.dt.float16

# test.py can flip these before calling kernel() to get an NTFF profile.
TRACE = False
LAST_RESULT = None

# store block sizes per group: halves, but quarters for the last group
STORES = [[256, 256], [256, 256], [256, 256], [128, 128, 128, 128]]


def _build(has_bias: bool, c1: float, c2: float) -> bass.Bass:
    nc = bacc.Bacc("TRN2", target_bir_lowering=False)
    xt = nc.dram_tensor("xt", [GROUPS, P, KCH, G], F16, kind="ExternalInput")
    wt = nc.dram_tensor("wt", [P, KCH, L], F16, kind="ExternalInput")
    if has_bias:
        bt = nc.dram_tensor("bt", [P, KCH], F32, kind="ExternalInput")
    # output in store blocks: for each group, contiguous [P, KCH, Hi] blocks
    yts = []
    for g, blocks in enumerate(STORES):
        yts.append(
            [
                nc.dram_tensor(f"yt{g}_{i}", [P, KCH, Hi], F16, kind="ExternalOutput")
                for i, Hi in enumerate(blocks)
            ]
        )

    with tile.TileContext(nc) as tc, ExitStack() as ctx:
        singles = ctx.enter_context(tc.tile_pool(name="singles", bufs=1))
        xpool = ctx.enter_context(tc.tile_pool(name="xpool", bufs=4))
        ypool = ctx.enter_context(tc.tile_pool(name="ypool", bufs=6))
        small = ctx.enter_context(tc.tile_pool(name="small", bufs=4))
        psT = ctx.enter_context(tc.tile_pool(name="psT", bufs=2, space="PSUM"))
        psB = ctx.enter_context(tc.tile_pool(name="psB", bufs=2, space="PSUM"))

        # ship W compact (6 KiB) on the SWDGE ring and spread it into the
        # zero-padded stationary layout on-device, so the preload does not
        # steal DMA-queue time from the first x in-DMAs
        wt3_sb = singles.tile([P, KCH, L], F16)
        nc.sync.dma_start(out=wt3_sb, in_=wt[:])
        wt_sb = singles.tile([P, KCH, LPAD], F16)
        nc.vector.memset(wt_sb, 0.0)
        nc.vector.tensor_copy(wt_sb[:, :, 0:LPAD:32], wt3_sb)
        w1_sb = singles.tile([1, LPAD], F16)
        nc.vector.memset(w1_sb, 0.0)
        for l in range(L):
            nc.vector.memset(w1_sb[:, 32 * l : 32 * l + 1], 1.0)
        one_row = singles.tile([1, G], F16)
        nc.vector.memset(one_row, 1.0)
        ones_col = singles.tile([1, P], F16)
        nc.vector.memset(ones_col, 1.0)
        if has_bias:
            bt_sb = singles.tile([P, KCH], F32)
            nc.sync.dma_start(out=bt_sb, in_=bt[:])

        KH = KCH // 2
        xs_t = [None] * GROUPS
        ah_t = [None] * GROUPS
        pb_t = [None] * GROUPS

        def emit_bcast(g):
            # pb[:, j] = a3[j] for all partitions: ones[1,P].T @ ah[1,G]
            pb = psB.tile([P, G], F32)
            nc.tensor.matmul(pb, ones_col, ah_t[g])
            pbh = small.tile([P, G], F16, tag="pbh")
            pb_t[g] = pbh
            nc.scalar.copy(out=pbh, in_=pb)

        def emit_stores(g):
            r = 0
            for i, Hi in enumerate(STORES[g]):
                ys = ypool.tile([P, KCH, Hi], F16, tag=f"ys{Hi}")
                pb = pb_t[g][:, r : r + Hi]
                pb_b = bass.AP(
                    tensor=pb.tensor,
                    offset=pb.offset,
                    ap=[pb.ap[0], [0, KCH], pb.ap[1]],
                )
                nc.vector.tensor_mul(ys, xs_t[g][:, :, r : r + Hi], pb_b)
                if has_bias:
                    for k in range(KCH):
                        nc.vector.tensor_scalar_add(
                            ys[:, k, :], ys[:, k, :], bt_sb[:, k : k + 1]
                        )
                # out-DMA on the ACT HWDGE ring
                nc.scalar.dma_start(out=yts[g][i][:], in_=ys)
                r += Hi

        for g in range(GROUPS):
            xs = xpool.tile([P, KCH, G], F16, tag="xs")
            xs_t[g] = xs
            # split per chunk-range so the first matmuls can start after
            # only part of the group has landed (finest for group 0)
            if g == 0:
                for k0 in range(0, KCH, 2):
                    nc.sync.dma_start(
                        out=xs[:, k0 : k0 + 2, :], in_=xt[g, :, k0 : k0 + 2, :]
                    )
            else:
                nc.sync.dma_start(out=xs[:, 0:KH, :], in_=xt[g, :, 0:KH, :])
                nc.sync.dma_start(out=xs[:, KH:KCH, :], in_=xt[g, :, KH:KCH, :])
            # U[32*l, j] = 1 + sum_d x[j, d] * W[l, d]; the +1 comes from a
            # ones-column matmul riding the same PSUM accumulation
            if g >= 1:
                # previous group's broadcast + stores, at high priority so
                # the tile scheduler does NOT sink them behind this group's
                # dot matmuls (which are input-paced and have slack)
                with tc.high_priority():
                    emit_bcast(g - 1)
                    emit_stores(g - 1)
            pt = psT.tile([LPAD, G], F32)
            for k in range(KCH):
                nc.tensor.matmul(
                    pt, wt_sb[:, k, :], xs[:, k, :], start=(k == 0), stop=False
                )
            nc.tensor.matmul(pt, w1_sb, one_row, start=False, stop=True)
            # a3 = ((U0*U1)+c1)*U2 + c2  (c1 = c2 = 0 when bias is zero)
            ua = small.tile([1, G], F32, tag="ua")
            nc.scalar.copy(out=ua, in_=pt[0:1, :])
            a = small.tile([1, G], F32, tag="a")
            nc.vector.tensor_mul(a, ua, pt[32:33, :])
            if c1 != 0.0:
                nc.vector.tensor_scalar_add(a, a, c1)
            ah = small.tile([1, G], F16, tag="ah")
            if c2 != 0.0:
                a2 = small.tile([1, G], F32, tag="a2")
                nc.vector.tensor_mul(a2, a, pt[64:65, :])
                nc.vector.tensor_scalar_add(a2, a2, c2)
                nc.scalar.copy(out=ah, in_=a2)
            else:
                nc.vector.tensor_mul(ah, a, pt[64:65, :])
            ah_t[g] = ah
        # tail: last group's broadcast + stores (PE is idle now)
        emit_bcast(GROUPS - 1)
        emit_stores(GROUPS - 1)
    nc.finalize()
    return nc


def kernel(x, W, bias):
    global LAST_RESULT
    x2 = np.asarray(x, dtype=np.float32).reshape(B, D)
    W2 = np.asarray(W, dtype=np.float32).reshape(L, D)
    B2 = np.asarray(bias, dtype=np.float32).reshape(L, D)

    # host-side constants
    has_bias = bool(np.any(B2 != 0.0))
    c1 = float(B2[0] @ W2[1])
    c2 = float((B2[0] + B2[1]) @ W2[2])
    b3 = B2.sum(axis=0)
    # wt[p, k, l] = W[l, k*128 + p]
    wt_host = np.ascontiguousarray(
        W2.T.reshape(KCH, P, L).transpose(1, 0, 2).astype(np.float16)
    )
    # bt[p, k] = B_L[k*128 + p]
    bt_host = np.ascontiguousarray(b3.reshape(KCH, P).T)

    nc = _build(has_bias, c1 if has_bias else 0.0, c2 if has_bias else 0.0)

    in_maps = []
    for c in range(N_CORES):
        xc = x2[c * ROWS : (c + 1) * ROWS]
        # xt[g, p, k, j] = xc[g*G + j, k*128 + p]
        xt_host = np.ascontiguousarray(
            xc.reshape(GROUPS, G, KCH, P).transpose(0, 3, 2, 1).astype(np.float16)
        )
        m = {"xt": xt_host, "wt": wt_host}
        if has_bias:
            m["bt"] = bt_host
        in_maps.append(m)

    kwargs = {}
    if TRACE:
        kwargs = dict(trace=True, trace_cores=[0])
    res = run_bass_kernel_spmd(nc, in_maps, core_ids=list(range(N_CORES)), **kwargs)
    LAST_RESULT = res
    out = np.empty((B, D), dtype=np.float32)
    for c in range(N_CORES):
        row0 = 0
        for g, blocks in enumerate(STORES):
            for i, Hi in enumerate(blocks):
                yb = res.results[c]["yt%d_%d" % (g, i)]  # [P, KCH, Hi]
                # y[row0 + j, k*128 + p] = yb[p, k, j]
                out[c * ROWS + row0 : c * ROWS + row0 + Hi] = (
                    yb.transpose(2, 1, 0).reshape(Hi, D).astype(np.float32)
                )
                row0 += Hi
    return np.ascontiguousarray(out.reshape(B, D, 1))


# revision 17
# speedup vs baseline: 1.1033x; 1.1033x over previous
# DCN CrossLayer kernel for Trainium2 (8 NeuronCores, data-parallel over batch).
#
# Reference computation (per example row x of length D, L=3 layers):
#   cross = x
#   for i in range(L):
#       s_i   = <cross, W_i>                  (scalar per example)
#       cross = x * s_i + bias_i + cross
#
# Algebraic collapse: cross_i = a_i * x + B_i with per-example scalar a_i and
# batch-independent vector B_i = sum_{j<i} bias_j.  Then
#   s_i     = a_i * t_i + c_i,   t_i = <x, W_i>,  c_i = <B_i, W_i>
#   a_{i+1} = a_i * (1 + t_i) + c_i
#   out     = a_L * x + B_L
# so the device kernel only needs the three dot products t_i = <x, W_i>,
# a tiny per-row recurrence, and one per-row scale of x.  c_i and B_L are
# computed on the host (they do not depend on the batch).
#
# Measured HW facts this version is tuned around (perfetto):
#   - 16 DMA queues x ~25 GB/s each => ~400 GB/s/core aggregate; descriptor
#     cost scales with bytes, so fp16 I/O (4+4 MiB/core) floors DMA at ~21us
#   - PE matmul cost is ~flat (~215ns + 152ns ldweights) for N <= 512, so
#     dot-groups are 512 rows
#   - gpsimd partition_broadcast has multi-us ucode dispatch latency, so a3
#     is broadcast by a PE ones-matmul into PSUM instead, placed mid-way
#     through the NEXT group's dot matmuls (its operand is ready by then,
#     so the PE stream never stalls and ys can read pb straight from PSUM)
#   - the +1s ride the PSUM accumulation (ones-column matmul); recurrence
#     is one ACT pull of U0 plus two DVE muls reading U1/U2 straight from
#     PSUM at partitions 32/64 (quadrant rule; one PSUM operand per op)
#   - ys/store are emitted per 256-row half (last group: 128-row quarters
#     to shorten the end-of-kernel drip) with a one-group skew, so output
#     DMAs interleave with remaining input DMAs on the shared queues
import os
from contextlib import ExitStack

import numpy as np

import concourse.bacc as bacc
import concourse.bass as bass
import concourse.tile as tile
from concourse import mybir
from concourse.bass_utils import run_bass_kernel_spmd

B, D, L = 16384, 1024, 3
N_CORES = 8
ROWS = B // N_CORES  # rows per core
P = 128
KCH = D // P  # 8 d-chunks of 128
GROUPS = 4
G = ROWS // GROUPS  # 512 rows per dot-group
LPAD = 65  # zero-padded stationary width; layer l at column 32*l

F32 = mybir.dt.float32
F16 = mybir.dt.float16

# test.py can flip these before calling kernel() to get an NTFF profile.
TRACE = False
LAST_RESULT = None

# store block sizes per group: halves, but quarters for the last group
STORES = [[256, 256], [256, 256], [256, 256], [128, 128, 128, 128]]


def _build(has_bias: bool, c1: float, c2: float) -> bass.Bass:
    nc = bacc.Bacc("TRN2", target_bir_lowering=False)
    xt = nc.dram_tensor("xt", [GROUPS, P, KCH, G], F16, kind="ExternalInput")
    wt = nc.dram_tensor("wt", [P, KCH, L], F16, kind="ExternalInput")
    if has_bias:
        bt = nc.dram_tensor("bt", [P, KCH], F32, kind="ExternalInput")
    # output in store blocks: for each group, contiguous [P, KCH, Hi] blocks
    yts = []
    for g, blocks in enumerate(STORES):
        yts.append(
            [
                nc.dram_tensor(f"yt{g}_{i}", [P, KCH, Hi], F16, kind="ExternalOutput")
                for i, Hi in enumerate(blocks)
            ]
        )

    with tile.TileContext(nc) as tc, ExitStack() as ctx:
        singles = ctx.enter_context(tc.tile_pool(name="singles", bufs=1))
        xpool = ctx.enter_context(tc.tile_pool(name="xpool", bufs=4))
        ypool = ctx.enter_context(tc.tile_pool(name="ypool", bufs=6))
        small = ctx.enter_context(tc.tile_pool(name="small", bufs=4))
        psT = ctx.enter_context(tc.tile_pool(name="psT", bufs=2, space="PSUM"))
        psB = ctx.enter_context(tc.tile_pool(name="psB", bufs=2, space="PSUM"))

        # ship W compact (6 KiB) on the SWDGE ring and spread it into the
        # zero-padded stationary layout on-device, so the preload does not
        # steal DMA-queue time from the first x in-DMAs
        wt3_sb = singles.tile([P, KCH, L], F16)
        nc.gpsimd.dma_start(out=wt3_sb, in_=wt[:])
        wt_sb = singles.tile([P, KCH, LPAD], F16)
        nc.vector.memset(wt_sb, 0.0)
        nc.vector.tensor_copy(wt_sb[:, :, 0:LPAD:32], wt3_sb)
        w1_sb = singles.tile([1, LPAD], F16)
        nc.vector.memset(w1_sb, 0.0)
        for l in range(L):
            nc.vector.memset(w1_sb[:, 32 * l : 32 * l + 1], 1.0)
        one_row = singles.tile([1, G], F16)
        nc.vector.memset(one_row, 1.0)
        ones_col = singles.tile([1, P], F16)
        nc.vector.memset(ones_col, 1.0)
        if has_bias:
            bt_sb = singles.tile([P, KCH], F32)
            nc.gpsimd.dma_start(out=bt_sb, in_=bt[:])

        KH = KCH // 2
        xs_t = [None] * GROUPS
        ah_t = [None] * GROUPS
        pb_t = [None] * GROUPS

        def emit_bcast(g):
            # pb[:, j] = a3[j] for all partitions: ones[1,P].T @ ah[1,G]
            pb = psB.tile([P, G], F32)
            nc.tensor.matmul(pb, ones_col, ah_t[g])
            pbh = small.tile([P, G], F16, tag="pbh")
            pb_t[g] = pbh
            nc.scalar.copy(out=pbh, in_=pb)

        def emit_stores(g):
            r = 0
            for i, Hi in enumerate(STORES[g]):
                ys = ypool.tile([P, KCH, Hi], F16, tag=f"ys{Hi}")
                pb = pb_t[g][:, r : r + Hi]
                pb_b = bass.AP(
                    tensor=pb.tensor,
                    offset=pb.offset,
                    ap=[pb.ap[0], [0, KCH], pb.ap[1]],
                )
                nc.vector.tensor_mul(ys, xs_t[g][:, :, r : r + Hi], pb_b)
                if has_bias:
                    for k in range(KCH):
                        nc.vector.tensor_scalar_add(
                            ys[:, k, :], ys[:, k, :], bt_sb[:, k : k + 1]
                        )
                # out-DMA on the ACT HWDGE ring
                nc.scalar.dma_start(out=yts[g][i][:], in_=ys)
                r += Hi

        for g in range(GROUPS):
            xs = xpool.tile([P, KCH, G], F16, tag="xs")
            xs_t[g] = xs
            # split per chunk-range so the first matmuls can start after
            # only part of the group has landed (finest for group 0)
            if g == 0:
                for k0 in range(0, KCH, 2):
                    nc.sync.dma_start(
                        out=xs[:, k0 : k0 + 2, :], in_=xt[g, :, k0 : k0 + 2, :]
                    )
            else:
                nc.sync.dma_start(out=xs[:, 0:KH, :], in_=xt[g, :, 0:KH, :])
                nc.sync.dma_start(out=xs[:, KH:KCH, :], in_=xt[g, :, KH:KCH, :])
            # U[32*l, j] = 1 + sum_d x[j, d] * W[l, d]; the +1 comes from a
            # ones-column matmul riding the same PSUM accumulation
            if g >= 1:
                # previous group's broadcast + stores first: dots(g) are
                # input-paced anyway, so this launches the output pipeline
                # as early as possible without stalling the PE
                emit_bcast(g - 1)
                emit_stores(g - 1)
            pt = psT.tile([LPAD, G], F32)
            for k in range(KCH):
                nc.tensor.matmul(
                    pt, wt_sb[:, k, :], xs[:, k, :], start=(k == 0), stop=False
                )
            nc.tensor.matmul(pt, w1_sb, one_row, start=False, stop=True)
            # a3 = ((U0*U1)+c1)*U2 + c2  (c1 = c2 = 0 when bias is zero)
            ua = small.tile([1, G], F32, tag="ua")
            nc.scalar.copy(out=ua, in_=pt[0:1, :])
            a = small.tile([1, G], F32, tag="a")
            nc.vector.tensor_mul(a, ua, pt[32:33, :])
            if c1 != 0.0:
                nc.vector.tensor_scalar_add(a, a, c1)
            ah = small.tile([1, G], F16, tag="ah")
            if c2 != 0.0:
                a2 = small.tile([1, G], F32, tag="a2")
                nc.vector.tensor_mul(a2, a, pt[64:65, :])
                nc.vector.tensor_scalar_add(a2, a2, c2)
                nc.scalar.copy(out=ah, in_=a2)
            else:
                nc.vector.tensor_mul(ah, a, pt[64:65, :])
            ah_t[g] = ah
        # tail: last group's broadcast + stores (PE is idle now)
        emit_bcast(GROUPS - 1)
        emit_stores(GROUPS - 1)
    nc.finalize()
    return nc


def kernel(x, W, bias):
    global LAST_RESULT
    x2 = np.asarray(x, dtype=np.float32).reshape(B, D)
    W2 = np.asarray(W, dtype=np.float32).reshape(L, D)
    B2 = np.asarray(bias, dtype=np.float32).reshape(L, D)

    # host-side constants
    has_bias = bool(np.any(B2 != 0.0))
    c1 = float(B2[0] @ W2[1])
    c2 = float((B2[0] + B2[1]) @ W2[2])
    b3 = B2.sum(axis=0)
    # wt[p, k, l] = W[l, k*128 + p]
    wt_host = np.ascontiguousarray(
        W2.T.reshape(KCH, P, L).transpose(1, 0, 2).astype(np.float16)
    )
    # bt[p, k] = B_L[k*128 + p]
    bt_host = np.ascontiguousarray(b3.reshape(KCH, P).T)

    nc = _build(has_bias, c1 if has_bias else 0.0, c2 if has_bias else 0.0)

    in_maps = []
    for c in range(N_CORES):
        xc = x2[c * ROWS : (c + 1) * ROWS]
        # xt[g, p, k, j] = xc[g*G + j, k*128 + p]
        xt_host = np.ascontiguousarray(
            xc.reshape(GROUPS, G, KCH, P).transpose(0, 3, 2, 1).astype(np.float16)
        )
        m = {"xt": xt_host, "wt": wt_host}
        if has_bias:
            m["bt"] = bt_host
        in_maps.append(m)

    kwargs = {}
    if TRACE:
        kwargs = dict(trace=True, trace_cores=[0])
    res = run_bass_kernel_spmd(nc, in_maps, core_ids=list(range(N_CORES)), **kwargs)
    LAST_RESULT = res
    out = np.empty((B, D), dtype=np.float32)
    for c in range(N_CORES):
        row0 = 0
        for g, blocks in enumerate(STORES):
            for i, Hi in enumerate(blocks):
                yb = res.results[c]["yt%d_%d" % (g, i)]  # [P, KCH, Hi]
                # y[row0 + j, k*128 + p] = yb[p, k, j]
                out[c * ROWS + row0 : c * ROWS + row0 + Hi] = (
                    yb.transpose(2, 1, 0).reshape(Hi, D).astype(np.float32)
                )
                row0 += Hi
    return np.ascontiguousarray(out.reshape(B, D, 1))
